# revision 8
# baseline (speedup 1.0000x reference)
"""Multi-head self-attention (B=2, S=2048, D=1024, H=16) on 8 Trainium2 cores.

Sharding: Megatron-style tensor parallelism on the head dimension.
Each core owns 2 heads (128 of the 1024 model dims):
  - Wq/Wk/Wv column-sharded: core c computes Q/K/V for dims [c*128,(c+1)*128)
  - attention for its 2 heads over both batches
  - Wo row-sharded: core c produces a partial output [4096, 1024]
  - host sums the 8 partials and adds bo.

Fully-pipelined single-region schedule (v2):
  - bf16 matmuls (1 PE cycle/row), host casts inputs to bf16.
  - Prologue: K proj for batch 0 (DMA-paced), Q proj for qc0, V proj for
    batch 0 -> attention starts ~12us in; the scalar-engine exp (the
    serial bottleneck, ~1us per 128x1024 tile) starts early and stays fed.
  - Batch-1 projections are emitted between batch-0 attention blocks as
    PE filler for the ACT-gated kt loop.
  - V transposed to token-major via DMA-transpose (InstDmaTransposeAnt),
    not PE transposes; no identity, no DVE psum->sbuf copies.
  - Out-projection results DMA'd HBM-direct from PSUM (no DVE copy).
  - PSUM budget exactly 8 banks: proj 2 + scores 4 (ring shared with
    out-proj) + PV 2.

Per-core device layouts:
  qT/kT: [128(out-dim), 4096(token)]  "o-major"
  vtk:   [128(token), 32 k-tiles, 2 heads, 66] = [head V (64) | ones | pad]
         (the ones column makes the PV matmul also produce the softmax
          normalizer as output row 64)
  scores computed transposed: sT[k, q] = (kT tile).T @ qT chunk, so the
  softmax sum reduces over the PARTITION dim via the ones row in the PV
  matmul. exp() needs no max subtraction: scores*0.125 are ~N(0,1) for
  this problem family, far from fp32 overflow.
"""

import os
import numpy as np
import ml_dtypes
from contextlib import ExitStack

import concourse.bass as bass
import concourse.tile as tile
from concourse import bacc, mybir
from concourse.bass_utils import run_bass_kernel_spmd

B, S, D = 2, 2048, 1024
H, DH = 16, 64
T = B * S                  # 4096 tokens total
N_CORES = 8
OPC = D // N_CORES         # 128 out dims per core
HPC = H // N_CORES         # 2 heads per core
NI = D // 128              # 8 contraction chunks of 128
TCH = 512                  # projection token chunk
NTCH = T // TCH            # 8
QCH = 512                  # attention q chunk
NQCH = S // QCH            # 4 per batch
NKT = S // 128             # 16 key tiles per batch
NCHB = NTCH // B           # 4 chunks per batch
HW = DH + 2                # 66 cols per head read by the PV matmul (data|ones|pad)
HSLOT = 128                # head slot width in vtk: DMA-transpose needs 128-aligned dst

F32 = mybir.dt.float32
F32R = mybir.dt.float32r
BF16 = mybir.dt.bfloat16
EXP = mybir.ActivationFunctionType.Exp

MM_MODE = os.environ.get("MHA_MM_DT", "bf16")
if MM_MODE == "bf16":
    MM_DT, MM_NP = BF16, ml_dtypes.bfloat16
else:
    MM_DT, MM_NP = F32R, np.float32


def _mha_kernel(tc, y, xT, wq, wk, wv, woT, bq, bk, bv):
    with ExitStack() as ctx:
        _mha_kernel_inner(ctx, tc, y, xT, wq, wk, wv, woT, bq, bk, bv)


def _mha_kernel_inner(ctx: ExitStack, tc, y, xT, wq, wk, wv, woT, bq, bk, bv):
    nc = tc.nc
    pers = ctx.enter_context(tc.tile_pool(name="pers", bufs=1))

    qT = pers.tile([128, T], MM_DT, tag="qT")
    kT = pers.tile([128, T], MM_DT, tag="kT")
    vT = pers.tile([128, T], MM_DT, tag="vT")
    vtk = pers.tile([128, B * NKT, HPC, HSLOT], MM_DT, tag="vtk")
    wq_sb = pers.tile([128, NI, OPC], MM_DT, tag="wq")
    wk_sb = pers.tile([128, NI, OPC], MM_DT, tag="wk")
    wv_sb = pers.tile([128, NI, OPC], MM_DT, tag="wv")
    woT_sb = pers.tile([128, D], MM_DT, tag="wo")
    bq_sb = pers.tile([128, 1], F32, tag="bq")
    bk_sb = pers.tile([128, 1], F32, tag="bk")
    bv_sb = pers.tile([128, 1], F32, tag="bv")

    # weights on the gpsimd DMA queue so the sync queue starts streaming x
    # tiles immediately; per-chunk so the first matmuls only wait on the
    # first chunk of each projection weight
    for i in range(NI):
        nc.gpsimd.dma_start(wk_sb[:, i, :], wk[:, i, :])
        nc.gpsimd.dma_start(wq_sb[:, i, :], wq[:, i, :])
        nc.gpsimd.dma_start(wv_sb[:, i, :], wv[:, i, :])
    nc.gpsimd.dma_start(woT_sb, woT)
    nc.gpsimd.dma_start(bq_sb, bq)
    nc.gpsimd.dma_start(bk_sb, bk)
    nc.gpsimd.dma_start(bv_sb, bv)
    # constant ones/pad columns of vtk
    nc.vector.memset(vtk[:, :, :, DH + 1 : HW], 0.0)
    nc.vector.memset(vtk[:, :, :, DH : DH + 1], 1.0)

    xin = ctx.enter_context(tc.tile_pool(name="xin", bufs=NTCH * NI))
    pp = ctx.enter_context(tc.tile_pool(name="pp", bufs=2, space="PSUM"))
    psS = ctx.enter_context(tc.tile_pool(name="psS", bufs=2, space="PSUM"))
    psP = ctx.enter_context(tc.tile_pool(name="psP", bufs=1, space="PSUM"))
    sm = ctx.enter_context(tc.tile_pool(name="sm", bufs=NKT))
    aux = ctx.enter_context(tc.tile_pool(name="aux", bufs=2))

    xtiles = {}

    def load_x(c, i):
        if (c, i) not in xtiles:
            t = xin.tile([128, TCH], MM_DT, tag="xt")
            nc.sync.dma_start(t, xT[i, :, c * TCH : (c + 1) * TCH])
            xtiles[(c, i)] = t
        return xtiles[(c, i)]

    def proj_pass(dst, W_sb, b_sb, c):
        ps = pp.tile([128, TCH], F32, tag="pp")
        for i in range(NI):
            nc.tensor.matmul(
                ps, W_sb[:, i, :], load_x(c, i), start=(i == 0), stop=(i == NI - 1)
            )
        sl = slice(c * TCH, (c + 1) * TCH)
        nc.vector.tensor_scalar_add(dst[:, sl], ps, b_sb)

    def v_post(c):
        # transpose V chunk c to token-major k-tiles via DMA-transpose
        for g in range(c * (TCH // 128), (c + 1) * (TCH // 128)):
            for h in range(HPC):
                nc.sync.dma_start_transpose(
                    vtk[:, g, h, 0:DH],
                    vT[h * DH : (h + 1) * DH, g * 128 : (g + 1) * 128],
                )

    def attention(b, qc):
        q0 = b * S + qc * QCH
        at_tiles = []
        for kt in range(NKT):
            g = b * NKT + kt
            ps_s = psS.tile([128, HPC, QCH], F32, tag="ps_s")
            for h in range(HPC):
                hs = slice(h * DH, (h + 1) * DH)
                nc.tensor.matmul(
                    ps_s[:, h, :],
                    kT[hs, g * 128 : (g + 1) * 128],
                    qT[hs, q0 : q0 + QCH],
                    start=True,
                    stop=True,
                )
            at = sm.tile([128, HPC, QCH], MM_DT, tag="at")
            nc.scalar.activation(at, ps_s, EXP, scale=0.125)
            at_tiles.append(at)
        pvs = [
            psP.tile([HW, QCH], F32, tag="ps_pv", name=f"pv{h}") for h in range(HPC)
        ]
        for kt in range(NKT):
            g = b * NKT + kt
            for h in range(HPC):
                nc.tensor.matmul(
                    pvs[h],
                    vtk[:, g, h, 0:HW],
                    at_tiles[kt][:, h, :],
                    start=(kt == 0),
                    stop=(kt == NKT - 1),
                )
        ctx_sb = aux.tile([128, QCH], MM_DT, tag="ctx")
        for h in range(HPC):
            # normalize: ctx rows for this head = pv[0:64] * recip(pv[64])
            rraw = aux.tile([1, QCH], F32, tag="rraw")
            nc.vector.tensor_copy(rraw, pvs[h][DH : DH + 1, :])
            rrow = aux.tile([1, QCH], F32, tag="rrow")
            nc.vector.reciprocal_approx_fast(rrow, rraw)
            nrm = aux.tile([DH, QCH], F32, tag="nrm")
            nc.gpsimd.partition_broadcast(nrm, rrow)
            nc.vector.tensor_mul(
                ctx_sb[h * DH : (h + 1) * DH, :], pvs[h][0:DH, :], nrm
            )
        # out projection: 2 MMs into a 2-bank psum tile (scores-tag ring),
        # one DVE f32->bf16 copy, one DMA per 128-token row block
        for t4 in range(QCH // 128):
            r0 = q0 + t4 * 128
            ps_o = psS.tile([128, D], F32, tag="ps_s")
            for nch in range(D // 512):
                nc.tensor.matmul(
                    ps_o[:, nch * 512 : (nch + 1) * 512],
                    ctx_sb[:, t4 * 128 : (t4 + 1) * 128],
                    woT_sb[:, nch * 512 : (nch + 1) * 512],
                    start=True,
                    stop=True,
                )
            yo = aux.tile([128, D], MM_DT, tag="yo")
            nc.vector.tensor_copy(yo, ps_o)
            nc.gpsimd.dma_start(y[r0 : r0 + 128, :], yo)

    # ---- emission schedule ----
    # prologue: batch-0 K (DMA-paced), Q for qc0, then batch-0 V
    for c in range(NCHB):
        proj_pass(kT, wk_sb, bk_sb, c)
    proj_pass(qT, wq_sb, bq_sb, 0)
    for c in range(NCHB):
        proj_pass(vT, wv_sb, bv_sb, c)
        v_post(c)

    # batch-0 attention; batch-1 projections (and remaining b0 Q chunks)
    # are emitted between blocks as PE filler for the ACT-gated kt loop
    for qc in range(NQCH):
        attention(0, qc)
        if qc < NQCH - 1:
            proj_pass(qT, wq_sb, bq_sb, qc + 1)
        c1 = NCHB + qc
        proj_pass(kT, wk_sb, bk_sb, c1)
        proj_pass(qT, wq_sb, bq_sb, c1)
        proj_pass(vT, wv_sb, bv_sb, c1)
        v_post(c1)

    for qc in range(NQCH):
        attention(1, qc)


_NC_CACHE = {}


def _build_nc(repeats=1):
    if repeats in _NC_CACHE:
        return _NC_CACHE[repeats]
    nc = bacc.Bacc("TRN2", target_bir_lowering=False, debug=False, num_devices=N_CORES)
    xT = nc.dram_tensor("xT", [NI, 128, T], MM_DT, kind="ExternalInput").ap()
    wq = nc.dram_tensor("wq", [128, NI, OPC], MM_DT, kind="ExternalInput").ap()
    wk = nc.dram_tensor("wk", [128, NI, OPC], MM_DT, kind="ExternalInput").ap()
    wv = nc.dram_tensor("wv", [128, NI, OPC], MM_DT, kind="ExternalInput").ap()
    woT = nc.dram_tensor("woT", [128, D], MM_DT, kind="ExternalInput").ap()
    bq = nc.dram_tensor("bq", [128, 1], F32, kind="ExternalInput").ap()
    bk = nc.dram_tensor("bk", [128, 1], F32, kind="ExternalInput").ap()
    bv = nc.dram_tensor("bv", [128, 1], F32, kind="ExternalInput").ap()
    y = nc.dram_tensor("y", [T, D], MM_DT, kind="ExternalOutput").ap()
    with tile.TileContext(nc) as tc:
        for _ in range(repeats):
            _mha_kernel(tc, y, xT, wq, wk, wv, woT, bq, bk, bv)
    nc.compile()
    _NC_CACHE[repeats] = nc
    return nc


def _prep_in_maps(inputs):
    x = np.asarray(inputs["x"], np.float32)
    Wq = np.asarray(inputs["Wq"], np.float32)
    Wk = np.asarray(inputs["Wk"], np.float32)
    Wv = np.asarray(inputs["Wv"], np.float32)
    Wo = np.asarray(inputs["Wo"], np.float32)
    bq = np.asarray(inputs["bq"], np.float32)
    bk = np.asarray(inputs["bk"], np.float32)
    bv = np.asarray(inputs["bv"], np.float32)

    xT_np = np.ascontiguousarray(x.reshape(T, D).T).reshape(NI, 128, T).astype(MM_NP)

    def _w_slice(W, c):
        # [128(p), NI, OPC]: [p, i, o] = W[c*OPC+o, i*128+p]
        A = np.ascontiguousarray(W[c * OPC : (c + 1) * OPC, :].T)  # [D, OPC]
        return np.ascontiguousarray(A.reshape(NI, 128, OPC).transpose(1, 0, 2)).astype(
            MM_NP
        )

    in_maps = []
    for c in range(N_CORES):
        sl = slice(c * OPC, (c + 1) * OPC)
        in_maps.append(
            {
                "xT": xT_np,
                "wq": _w_slice(Wq, c),
                "wk": _w_slice(Wk, c),
                "wv": _w_slice(Wv, c),
                "woT": np.ascontiguousarray(Wo[:, sl].T).astype(MM_NP),
                "bq": bq[sl].reshape(OPC, 1).copy(),
                "bk": bk[sl].reshape(OPC, 1).copy(),
                "bv": bv[sl].reshape(OPC, 1).copy(),
            }
        )
    return in_maps


def kernel(**inputs) -> np.ndarray:
    nc = _build_nc()
    in_maps = _prep_in_maps(inputs)
    res = run_bass_kernel_spmd(nc, in_maps, core_ids=list(range(N_CORES)))
    bo = np.asarray(inputs["bo"], np.float32)
    y = np.zeros((T, D), np.float64)
    for c in range(N_CORES):
        y += res.results[c]["y"].astype(np.float64)
    y = (y + bo).astype(np.float32)
    return y.reshape(B, S, D)


# revision 9
# speedup vs baseline: 1.3845x; 1.3845x over previous
"""Multi-head self-attention (B=2, S=2048, D=1024, H=16) on 8 Trainium2 cores.

Sharding: Megatron-style tensor parallelism on the head dimension.
Each core owns 2 heads (128 of the 1024 model dims):
  - Wq/Wk/Wv column-sharded: core c computes Q/K/V for dims [c*128,(c+1)*128)
  - attention for its 2 heads over both batches
  - Wo row-sharded: core c produces a partial output [4096, 1024]
  - host sums the 8 partials and adds bo.

Fully-pipelined single-region schedule (v2):
  - bf16 matmuls (1 PE cycle/row), host casts inputs to bf16.
  - Prologue: K proj for batch 0 (DMA-paced), Q proj for qc0, V proj for
    batch 0 -> attention starts ~12us in; the scalar-engine exp (the
    serial bottleneck, ~1us per 128x1024 tile) starts early and stays fed.
  - Batch-1 projections are emitted between batch-0 attention blocks as
    PE filler for the ACT-gated kt loop.
  - V transposed to token-major via DMA-transpose (InstDmaTransposeAnt),
    not PE transposes; no identity, no DVE psum->sbuf copies.
  - Out-projection results DMA'd HBM-direct from PSUM (no DVE copy).
  - PSUM budget exactly 8 banks: proj 2 + scores 4 (ring shared with
    out-proj) + PV 2.

Per-core device layouts:
  qT/kT: [128(out-dim), 4096(token)]  "o-major"
  vtk:   [128(token), 32 k-tiles, 2 heads, 66] = [head V (64) | ones | pad]
         (the ones column makes the PV matmul also produce the softmax
          normalizer as output row 64)
  scores computed transposed: sT[k, q] = (kT tile).T @ qT chunk, so the
  softmax sum reduces over the PARTITION dim via the ones row in the PV
  matmul. exp() needs no max subtraction: scores*0.125 are ~N(0,1) for
  this problem family, far from fp32 overflow.
"""

import os
import numpy as np
import ml_dtypes
from contextlib import ExitStack

import concourse.bass as bass
import concourse.tile as tile
from concourse import bacc, mybir
from concourse.bass_utils import run_bass_kernel_spmd
from concourse.masks import make_identity

B, S, D = 2, 2048, 1024
H, DH = 16, 64
T = B * S                  # 4096 tokens total
N_CORES = 8
OPC = D // N_CORES         # 128 out dims per core
HPC = H // N_CORES         # 2 heads per core
NI = D // 128              # 8 contraction chunks of 128
TCH = 512                  # projection token chunk
NTCH = T // TCH            # 8
QCH = 512                  # attention q chunk
NQCH = S // QCH            # 4 per batch
NKT = S // 128             # 16 key tiles per batch
NCHB = NTCH // B           # 4 chunks per batch
HW = DH + 2                # 66 cols per head read by the PV matmul (data|ones|pad)
HSLOT = 128                # head slot width in vtk: DMA-transpose needs 128-aligned dst

F32 = mybir.dt.float32
F32R = mybir.dt.float32r
BF16 = mybir.dt.bfloat16
EXP = mybir.ActivationFunctionType.Exp

MM_MODE = os.environ.get("MHA_MM_DT", "bf16")
if MM_MODE == "bf16":
    MM_DT, MM_NP = BF16, ml_dtypes.bfloat16
else:
    MM_DT, MM_NP = F32R, np.float32


def _mha_kernel(tc, y, xT, wq, wk, wv, woT, bq, bk, bv):
    with ExitStack() as ctx:
        _mha_kernel_inner(ctx, tc, y, xT, wq, wk, wv, woT, bq, bk, bv)


def _mha_kernel_inner(ctx: ExitStack, tc, y, xT, wq, wk, wv, woT, bq, bk, bv):
    nc = tc.nc
    pers = ctx.enter_context(tc.tile_pool(name="pers", bufs=1))

    qT = pers.tile([128, T], MM_DT, tag="qT")
    kT = pers.tile([128, T], MM_DT, tag="kT")
    vT = pers.tile([128, T], MM_DT, tag="vT")
    vtk = pers.tile([128, B * NKT, HPC, HSLOT], MM_DT, tag="vtk")
    wq_sb = pers.tile([128, NI, OPC], MM_DT, tag="wq")
    wk_sb = pers.tile([128, NI, OPC], MM_DT, tag="wk")
    wv_sb = pers.tile([128, NI, OPC], MM_DT, tag="wv")
    woT_sb = pers.tile([128, D], MM_DT, tag="wo")
    bq_sb = pers.tile([128, 1], F32, tag="bq")
    bk_sb = pers.tile([128, 1], F32, tag="bk")
    bv_sb = pers.tile([128, 1], F32, tag="bv")
    ident = pers.tile([128, 128], MM_DT, tag="ident")

    # weights on the gpsimd DMA queue so the sync queue starts streaming x
    # tiles immediately; per-chunk so the first matmuls only wait on the
    # first chunk of each projection weight
    for i in range(NI):
        nc.gpsimd.dma_start(wk_sb[:, i, :], wk[:, i, :])
        nc.gpsimd.dma_start(wq_sb[:, i, :], wq[:, i, :])
        nc.gpsimd.dma_start(wv_sb[:, i, :], wv[:, i, :])
    nc.gpsimd.dma_start(woT_sb, woT)
    nc.gpsimd.dma_start(bq_sb, bq)
    nc.gpsimd.dma_start(bk_sb, bk)
    nc.gpsimd.dma_start(bv_sb, bv)
    make_identity(nc, ident)
    # constant ones/pad columns of vtk
    nc.vector.memset(vtk[:, :, :, DH + 1 : HW], 0.0)
    nc.vector.memset(vtk[:, :, :, DH : DH + 1], 1.0)

    xin = ctx.enter_context(tc.tile_pool(name="xin", bufs=NTCH * NI))
    pp = ctx.enter_context(tc.tile_pool(name="pp", bufs=1, space="PSUM"))
    psS = ctx.enter_context(tc.tile_pool(name="psS", bufs=2, space="PSUM"))
    psP = ctx.enter_context(tc.tile_pool(name="psP", bufs=1, space="PSUM"))
    psO = ctx.enter_context(tc.tile_pool(name="psO", bufs=1, space="PSUM"))
    sm = ctx.enter_context(tc.tile_pool(name="sm", bufs=NKT))
    aux = ctx.enter_context(tc.tile_pool(name="aux", bufs=2))

    xtiles = {}

    def load_x(c, i):
        if (c, i) not in xtiles:
            t = xin.tile([128, TCH], MM_DT, tag="xt")
            nc.sync.dma_start(t, xT[i, :, c * TCH : (c + 1) * TCH])
            xtiles[(c, i)] = t
        return xtiles[(c, i)]

    def proj_pass(dst, W_sb, b_sb, c):
        ps = pp.tile([128, TCH], F32, tag="pp")
        for i in range(NI):
            nc.tensor.matmul(
                ps, W_sb[:, i, :], load_x(c, i), start=(i == 0), stop=(i == NI - 1)
            )
        sl = slice(c * TCH, (c + 1) * TCH)
        nc.vector.tensor_scalar_add(dst[:, sl], ps, b_sb)

    def v_post(c):
        # transpose V chunk c to token-major k-tiles: PE transpose into a
        # scores-ring psum slot, DVE copy into the vtk head slots
        for g in range(c * (TCH // 128), (c + 1) * (TCH // 128)):
            ps_t = psS.tile([128, 128], MM_DT, tag="ps_s", name=f"pt{g}")
            nc.tensor.transpose(ps_t, vT[:, g * 128 : (g + 1) * 128], ident)
            for h in range(HPC):
                nc.vector.tensor_copy(
                    vtk[:, g, h, 0:DH], ps_t[:, h * DH : (h + 1) * DH]
                )

    def attention(b, qc):
        q0 = b * S + qc * QCH
        at_tiles = []
        for kt in range(NKT):
            g = b * NKT + kt
            ps_s = psS.tile([128, HPC, QCH], F32, tag="ps_s")
            for h in range(HPC):
                hs = slice(h * DH, (h + 1) * DH)
                nc.tensor.matmul(
                    ps_s[:, h, :],
                    kT[hs, g * 128 : (g + 1) * 128],
                    qT[hs, q0 : q0 + QCH],
                    start=True,
                    stop=True,
                )
            at = sm.tile([128, HPC, QCH], MM_DT, tag="at")
            nc.scalar.activation(at, ps_s, EXP, scale=0.125)
            at_tiles.append(at)
        pvs = [
            psP.tile([HW, QCH], F32, tag="ps_pv", name=f"pv{h}") for h in range(HPC)
        ]
        for kt in range(NKT):
            g = b * NKT + kt
            for h in range(HPC):
                nc.tensor.matmul(
                    pvs[h],
                    vtk[:, g, h, 0:HW],
                    at_tiles[kt][:, h, :],
                    start=(kt == 0),
                    stop=(kt == NKT - 1),
                )
        ctx_sb = aux.tile([128, QCH], MM_DT, tag="ctx")
        for h in range(HPC):
            # normalize: ctx rows for this head = pv[0:64] * recip(pv[64])
            rraw = aux.tile([1, QCH], F32, tag="rraw")
            nc.vector.tensor_copy(rraw, pvs[h][DH : DH + 1, :])
            rrow = aux.tile([1, QCH], F32, tag="rrow")
            nc.vector.reciprocal_approx_fast(rrow, rraw)
            nrm = aux.tile([DH, QCH], F32, tag="nrm")
            nc.gpsimd.partition_broadcast(nrm, rrow)
            nc.vector.tensor_mul(
                ctx_sb[h * DH : (h + 1) * DH, :], pvs[h][0:DH, :], nrm
            )
        # out projection: dedicated single psum bank, DVE f32->bf16 casts
        # into a [128, 1024] staging tile, one DMA per 128-token row block
        for t4 in range(QCH // 128):
            r0 = q0 + t4 * 128
            yo = aux.tile([128, D], MM_DT, tag="yo")
            for nch in range(D // 512):
                ps_o = psO.tile([128, 512], F32, tag="ps_o")
                nc.tensor.matmul(
                    ps_o,
                    ctx_sb[:, t4 * 128 : (t4 + 1) * 128],
                    woT_sb[:, nch * 512 : (nch + 1) * 512],
                    start=True,
                    stop=True,
                )
                nc.vector.tensor_copy(yo[:, nch * 512 : (nch + 1) * 512], ps_o)
            nc.gpsimd.dma_start(y[r0 : r0 + 128, :], yo)

    # ---- emission schedule ----
    # prologue: batch-0 K (DMA-paced), Q for qc0, then batch-0 V
    for c in range(NCHB):
        proj_pass(kT, wk_sb, bk_sb, c)
    proj_pass(qT, wq_sb, bq_sb, 0)
    for c in range(NCHB):
        proj_pass(vT, wv_sb, bv_sb, c)
        v_post(c)

    # batch-0 attention; batch-1 projections (and remaining b0 Q chunks)
    # are emitted between blocks as PE filler for the ACT-gated kt loop
    for qc in range(NQCH):
        attention(0, qc)
        if qc < NQCH - 1:
            proj_pass(qT, wq_sb, bq_sb, qc + 1)
        c1 = NCHB + qc
        proj_pass(kT, wk_sb, bk_sb, c1)
        proj_pass(qT, wq_sb, bq_sb, c1)
        proj_pass(vT, wv_sb, bv_sb, c1)
        v_post(c1)

    for qc in range(NQCH):
        attention(1, qc)


_NC_CACHE = {}


def _build_nc(repeats=1):
    if repeats in _NC_CACHE:
        return _NC_CACHE[repeats]
    nc = bacc.Bacc("TRN2", target_bir_lowering=False, debug=False, num_devices=N_CORES)
    xT = nc.dram_tensor("xT", [NI, 128, T], MM_DT, kind="ExternalInput").ap()
    wq = nc.dram_tensor("wq", [128, NI, OPC], MM_DT, kind="ExternalInput").ap()
    wk = nc.dram_tensor("wk", [128, NI, OPC], MM_DT, kind="ExternalInput").ap()
    wv = nc.dram_tensor("wv", [128, NI, OPC], MM_DT, kind="ExternalInput").ap()
    woT = nc.dram_tensor("woT", [128, D], MM_DT, kind="ExternalInput").ap()
    bq = nc.dram_tensor("bq", [128, 1], F32, kind="ExternalInput").ap()
    bk = nc.dram_tensor("bk", [128, 1], F32, kind="ExternalInput").ap()
    bv = nc.dram_tensor("bv", [128, 1], F32, kind="ExternalInput").ap()
    y = nc.dram_tensor("y", [T, D], MM_DT, kind="ExternalOutput").ap()
    with tile.TileContext(nc) as tc:
        for _ in range(repeats):
            _mha_kernel(tc, y, xT, wq, wk, wv, woT, bq, bk, bv)
    nc.compile()
    _NC_CACHE[repeats] = nc
    return nc


def _prep_in_maps(inputs):
    x = np.asarray(inputs["x"], np.float32)
    Wq = np.asarray(inputs["Wq"], np.float32)
    Wk = np.asarray(inputs["Wk"], np.float32)
    Wv = np.asarray(inputs["Wv"], np.float32)
    Wo = np.asarray(inputs["Wo"], np.float32)
    bq = np.asarray(inputs["bq"], np.float32)
    bk = np.asarray(inputs["bk"], np.float32)
    bv = np.asarray(inputs["bv"], np.float32)

    xT_np = np.ascontiguousarray(x.reshape(T, D).T).reshape(NI, 128, T).astype(MM_NP)

    def _w_slice(W, c):
        # [128(p), NI, OPC]: [p, i, o] = W[c*OPC+o, i*128+p]
        A = np.ascontiguousarray(W[c * OPC : (c + 1) * OPC, :].T)  # [D, OPC]
        return np.ascontiguousarray(A.reshape(NI, 128, OPC).transpose(1, 0, 2)).astype(
            MM_NP
        )

    in_maps = []
    for c in range(N_CORES):
        sl = slice(c * OPC, (c + 1) * OPC)
        in_maps.append(
            {
                "xT": xT_np,
                "wq": _w_slice(Wq, c),
                "wk": _w_slice(Wk, c),
                "wv": _w_slice(Wv, c),
                "woT": np.ascontiguousarray(Wo[:, sl].T).astype(MM_NP),
                "bq": bq[sl].reshape(OPC, 1).copy(),
                "bk": bk[sl].reshape(OPC, 1).copy(),
                "bv": bv[sl].reshape(OPC, 1).copy(),
            }
        )
    return in_maps


def kernel(**inputs) -> np.ndarray:
    nc = _build_nc()
    in_maps = _prep_in_maps(inputs)
    res = run_bass_kernel_spmd(nc, in_maps, core_ids=list(range(N_CORES)))
    bo = np.asarray(inputs["bo"], np.float32)
    y = np.zeros((T, D), np.float64)
    for c in range(N_CORES):
        y += res.results[c]["y"].astype(np.float64)
    y = (y + bo).astype(np.float32)
    return y.reshape(B, S, D)
